# revision 1
# baseline (speedup 1.0000x reference)
"""Trainium2 Bass kernel for AudioAttentionMapGenerator.

Math (reference):
    sigma = exp(log_sigma); c = 0.5 / (sigma^2 + 1e-6)
    w_n   = attn_weights * mask
    map[b,h,w] = sum_n w_n * exp(-c*((h-v_bn)^2 + (w-u_bn)^2))
    out = map / (max_hw(map) + 1e-6)

Structure per core (2 samples, data-parallel over B=16 on 8 cores):
  - Separable Gaussians: map = Gy^T @ (w*Gx), two bf16 matmuls per sample
    with H chunked 128+96 (stationary limit 128).  bf16 runs the PE at
    1 cycle/row with no 256-wide f32r padding, at ~1e-3 rel error.
  - Gaussian phase: exp(-c(x-u)^2) = Exp(cneg*(gm2*u+grid2) + cneg*u^2)
    — one DVE STT + one ACT Exp per (sample, axis); weights then one
    bf16 2x-mode DVE multiply on the moving operand.  All four STTs sit
    on DVE (Pool's ISA rejects per-partition-scalar STTs, and folding
    ln(w) into the Exp bias is a trap: Ln lives in a different ACT table
    set and each table swap costs 1283ns).  ddx0 is ordered FIRST in the
    DVE queue (the first Exp gates on it), and explicit no-sync deps pin
    every queue order the Tile scheduler would otherwise shuffle.
  - Normalization: fused free-dim reduce_max (DVE) per sample over the
    whole PSUM map, partition all-reduce (GPSIMD), reciprocal (DVE),
    then scale0 on ACT and scale1 on DVE so both scales overlap and the
    last one follows its reciprocal with no cross-engine hop.
  - Output leaves via ONE kv_writeback prepared ~5us early, during the
    input-DMA latency window (prepare_only), and fired by trigger_dma
    after both scales: the tail is Pool-dispatch + 82ns transfer + sem,
    no HWDGE hold and no DGE->DMA latency.  The staging buffer uses two
    aliased SBUF tensor names (scales write st_w, descriptors read st_r)
    because Tile pins a prep's staging-read to the DMA completion and
    would otherwise WAR-deadlock the scales; real ordering is enforced
    by explicit trigger deps.  Staging layout [128 partitions, 4 chunks,
    224], chunk = 128 consecutive rows of the (2*256,224) row-padded
    output; garbage partitions land in pad rows, discarded host-side.
  - PE warm-up matmuls bridge from kernel start to the real matmuls so
    they hit the ramped PE clock; Tile's unsatisfiable epilogue waits on
    the SWDGE auto-bump sem are stripped (kernel end is still gated on
    the true DMA-completion sem via the Pool queue).
"""

import sys

import numpy as np

if "/opt/trn_rl_repo" not in sys.path:
    sys.path.insert(0, "/opt/trn_rl_repo")

B, N, H, W = 16, 128, 224, 224
NCORES = 8
BPC = B // NCORES  # samples per core
HP = 256  # padded rows per sample in DRAM (2 chunks of 128)

_CACHE = {}


def _build():
    if "nc" in _CACHE:
        return _CACHE["nc"]

    from contextlib import ExitStack

    import concourse.bass_isa as bass_isa
    import concourse.tile as tile
    from concourse import bacc, mybir
    from concourse.tile import add_dep_helper

    f32 = mybir.dt.float32
    bf16 = mybir.dt.bfloat16
    i32 = mybir.dt.int32
    AF = mybir.ActivationFunctionType
    AX = mybir.AxisListType
    OP = mybir.AluOpType

    nc = bacc.Bacc(
        "TRN2",
        target_bir_lowering=False,
        debug=False,
        enable_asserts=False,
        num_devices=NCORES,
    )
    # packed per-core input: [u0,u1,v0,v1, aw0,aw1, m0,m1, log_sigma, pad...]
    pk = nc.dram_tensor("pk", (N, 12), f32, kind="ExternalInput").ap()
    # per-sample rows padded 224->256 so every 128-row writeback chunk stays
    # in-bounds; host discards rows 224:256 of each sample
    out = nc.dram_tensor("out", (BPC * HP, W), f32, kind="ExternalOutput")

    with ExitStack() as ctx:
        tc = ctx.enter_context(tile.TileContext(nc))
        consts = ctx.enter_context(tc.tile_pool(name="consts", bufs=1))
        psum = ctx.enter_context(tc.tile_pool(name="psum", bufs=1, space="PSUM"))

        # ---- input DMA (lands ~2.9us in; everything below overlaps it) ----
        pkt = consts.tile([128, 12], f32)
        nc.sync.dma_start(out=pkt, in_=pk)

        # PE warm-up: dependency-free matmuls from kernel start keep the PE
        # in its continuous-busy ramp so the real matmuls hit the warm clock
        wseed = consts.tile([1, 64], f32)
        nc.vector.memset(wseed, 1.0)
        pwarm = psum.tile([1, 64], f32, tag="pwarm")
        for _ in range(17):
            nc.tensor.matmul(pwarm, wseed[0:1, 0:1], wseed[0:1, 0:64])

        # grid rows generated on-chip (no DMA involved): grid = x via one
        # prefix scan, then gm2 = -2x and grid2 = x^2 as elementwise ops —
        # the DVE head must clear before the input lands (~2.92us) or ddx0
        # becomes queue-gated instead of input-gated
        ones = consts.tile([128, W], f32)
        nc.vector.memset(ones, 1.0)
        grid = consts.tile([128, W], f32)
        nc.vector.tensor_tensor_scan(grid, ones, ones, -1.0, OP.add, OP.mult)
        gm2 = consts.tile([128, W], f32)
        nc.vector.tensor_scalar_mul(gm2, grid, -2.0)
        grid2 = consts.tile([128, W], f32)
        nc.vector.tensor_mul(grid2, grid, grid)

        # ---- output writeback: descriptors prepared NOW, fired at the end.
        # out viewed as [batch=4 chunks, 128 rows, dho=1, 224]; staging is
        # [128 partitions, 4 chunks, 224] (chunk c partition p = row 128c+p).
        # Staging buffer under two aliased names (same SBUF bytes): the
        # scales write st_w, the prep's descriptors read st_r.  Tile pins a
        # prep's staging-read to the DMA completion, so if the prep named
        # st_w the scales would deadlock on a WAR against the in-flight DMA;
        # with the alias the real ordering (scales -> trigger) is enforced
        # explicitly below and the prep can run ~5us early, off the tail.
        st_w = nc.alloc_sbuf_tensor("st_w", (128, 4, W), f32)
        st_r = nc.alloc_sbuf_tensor_at(
            "st_r", (128, 4, W), f32, offset=nc.lookup_mloc(st_w).addr
        )
        st = st_w.ap()
        ctxi = consts.tile([128, 4], i32)
        nc.gpsimd.memset(ctxi, 0)
        dma_sem = nc.alloc_semaphore("kv_dma")
        out4d = out.ap().rearrange("(c p) (d w) -> c p d w", p=128, d=1)
        nc.gpsimd.kv_writeback(
            out4d,
            st_r.ap().rearrange("p (d b) w -> p d b w", d=1),
            ctxi[:, :],
            prepare_only=True,
            sem=dma_sem,
        )

        # PSUM maps: [128, chunk, 224]; chunk1 only has 96 valid partitions,
        # zero the rest once so the fused reduce_max sees no garbage
        pmaps = []
        for b in range(BPC):
            pm = psum.tile([128, 2, W], f32, tag=f"pmap{b}")
            nc.vector.memset(pm[96:128, 1, :], 0.0)
            pmaps.append(pm)

        # ---- tiny input-dependent chain ----
        # cneg = -0.5/(sig^2+1e-6) = reciprocal((sig2+1e-6)*-2); the affine
        # step runs on ACT (a [128,1] Copy is ~free there) so DVE only pays
        # for the reciprocal before bias4, pulling every Exp ~75ns earlier
        sig2 = consts.tile([128, 1], f32)
        nc.scalar.activation(sig2, pkt[:, 8:9], AF.Exp, scale=2.0)
        sig2e = consts.tile([128, 1], f32)
        nc.scalar.activation(sig2e, sig2, AF.Copy, scale=-2.0, bias=-2e-6)
        cneg = consts.tile([128, 1], f32)
        i_cneg = nc.vector.reciprocal(cneg, sig2e)
        # w = attn*mask on GPSIMD (the Ln-into-Exp-bias folding is a trap:
        # Ln lives in a different ACT table set than Exp, and each table
        # swap costs 1283ns on the critical path)
        wt = consts.tile([128, BPC], f32)
        nc.gpsimd.tensor_mul(wt, pkt[:, 4:6], pkt[:, 6:8])

        # exponent terms: per-partition-scalar STTs only run on DVE (Pool's
        # ISA rejects TensorScalarPtr), so x/y both go there, interleaved in
        # the order the ACT Exps will consume them
        bias4 = consts.tile([128, 4], f32)
        i_bias4 = nc.vector.scalar_tensor_tensor(
            bias4, pkt[:, 0:4], cneg[:, 0:1], pkt[:, 0:4], OP.mult, OP.mult
        )
        # DVE queue order: ddx0 FIRST (it has no dependency on the cneg
        # chain, and the first Exp is gated on it), then the tiny chain,
        # then the remaining STTs in ACT-consumption order
        dds, ddys = [], []
        prev = None
        for b in range(BPC):
            ddx = consts.tile([128, W], f32, tag=f"ddx{b}")
            i_ddx = nc.vector.scalar_tensor_tensor(
                ddx, gm2, pkt[:, b : b + 1], grid2, OP.mult, OP.add
            )
            dds.append(ddx)
            ddy = consts.tile([128, W], f32, tag=f"ddy{b}")
            i_ddy = nc.vector.scalar_tensor_tensor(
                ddy, gm2, pkt[:, 2 + b : 3 + b], grid2, OP.mult, OP.add
            )
            ddys.append(ddy)
            if b == 0:
                # tiny chain right after ddx0, before ddy0
                add_dep_helper(i_cneg.ins, i_ddx.ins, sync=False, reason="ddx0 1st")
                add_dep_helper(i_ddy.ins, i_bias4.ins, sync=False, reason="bias b4 y")
            else:
                add_dep_helper(i_ddx.ins, prev.ins, sync=False, reason="stt order")
                add_dep_helper(i_ddy.ins, i_ddx.ins, sync=False, reason="stt order")
            prev = i_ddy

        # ---- Gaussian tiles (ACT queue: gx0, gy0, gx1, gy1) ----
        gxs, gys = [], []
        for b in range(BPC):
            gx = consts.tile([128, W], bf16, tag=f"gx{b}")
            nc.scalar.activation(
                gx, dds[b], AF.Exp, scale=cneg[:, 0:1], bias=bias4[:, b : b + 1]
            )
            gy = consts.tile([128, W], bf16, tag=f"gy{b}")
            nc.scalar.activation(
                gy, ddys[b], AF.Exp, scale=cneg[:, 0:1], bias=bias4[:, 2 + b : 3 + b]
            )
            gxs.append(gx)
            gys.append(gy)
        # weights onto the moving operand (bf16 2x-mode DVE multiplies)
        wgxs, i_wgxs = [], []
        for b in range(BPC):
            wgx = consts.tile([128, W], bf16, tag=f"wgx{b}")
            i_w = nc.vector.tensor_scalar_mul(wgx, gxs[b], wt[:, b : b + 1])
            add_dep_helper(i_w.ins, prev.ins, sync=False, reason="after stts")
            i_wgxs.append(i_w)
            wgxs.append(wgx)

        # ---- matmuls: map chunks of 128/96 rows ----
        for b in range(BPC):
            nc.tensor.matmul(pmaps[b][:, 0, :], gys[b][:, 0:128], wgxs[b])
            nc.tensor.matmul(pmaps[b][0:96, 1, :], gys[b][:, 128:W], wgxs[b])

        # ---- per-sample normalization into the staging tile ----
        # Engine split tuned so the two samples' chains overlap: reduce0 on
        # the idle GPSIMD, reduce1 on DVE (its earliest start is gated by
        # sample 1's matmuls anyway); rs0/rs1 on DVE; scale0 on ACT and
        # scale1 on DVE so the two scales run concurrently and the last one
        # sits right after its reciprocal with no cross-engine hop.
        # (GPSIMD cannot touch PSUM, so both free-dim reduces live on DVE;
        # the partition all-reduces go to the idle GPSIMD)
        mrow0 = consts.tile([128, 1], f32, tag="mrow0")
        i_red0 = nc.vector.reduce_max(mrow0, pmaps[0][:, :, :], axis=AX.XY)
        for i_w in i_wgxs:
            add_dep_helper(i_red0.ins, i_w.ins, sync=False, reason="wgx first")
        mall0 = consts.tile([128, 1], f32, tag="mall0")
        nc.gpsimd.partition_all_reduce(
            mall0, mrow0, channels=128, reduce_op=bass_isa.ReduceOp.max
        )
        mrow1 = consts.tile([128, 1], f32, tag="mrow1")
        i_red1 = nc.vector.reduce_max(mrow1, pmaps[1][:, :, :], axis=AX.XY)
        add_dep_helper(i_red1.ins, i_red0.ins, sync=False, reason="red0 first")
        mall1 = consts.tile([128, 1], f32, tag="mall1")
        nc.gpsimd.partition_all_reduce(
            mall1, mrow1, channels=128, reduce_op=bass_isa.ReduceOp.max
        )
        # reference adds 1e-6 to the max before dividing; omitting it
        # changes the result by ~1e-6 relative (max is O(1) here)
        rs0 = consts.tile([128, 1], f32, tag="rs0")
        i_rs0 = nc.vector.reciprocal(rs0, mall0)
        add_dep_helper(i_rs0.ins, i_red1.ins, sync=False, reason="red1 first")
        rs1 = consts.tile([128, 1], f32, tag="rs1")
        i_rs1 = nc.vector.reciprocal(rs1, mall1)
        add_dep_helper(i_rs1.ins, i_rs0.ins, sync=False, reason="rs0 first")
        i_scales = [
            nc.scalar.mul(st[:, 0:2, :], pmaps[0][:, :, :], rs0[:, 0:1]),
            nc.vector.tensor_scalar_mul(st[:, 2:4, :], pmaps[1][:, :, :], rs1[:, 0:1]),
        ]

        # ---- fire the prepared writeback ----
        # the alias hides the staging RAW from Tile, so wire the trigger's
        # deps on the scale instructions explicitly
        trig = nc.gpsimd.trigger_dma(count=None)
        for s in i_scales:
            add_dep_helper(trig.ins, s.ins, sync=True, reason="st written")
        # pin the completion wait AFTER the trigger in the in-order Pool
        # queue (a bare sem wait has no data deps, so the scheduler would
        # otherwise float it earlier and deadlock the queue)
        wsem = nc.gpsimd.wait_ge(dma_sem, 16)
        add_dep_helper(wsem.ins, trig.ins, sync=False, reason="wait after fire")

    # Tile's epilogue waits on the prep's DMASW proc sem, which only the real
    # SWDGE hardware auto-bumps (no instruction in the program does, so both
    # simulators deadlock on it).  The Pool queue already gates kernel end on
    # the true DMA-completion sem (kv_dma >= 16) and every engine's final
    # barrier gates on Pool, so these waits are redundant — drop them.
    for block in nc.m.functions[0].blocks:
        for ins in block.instructions:
            si = ins.sync_info
            if si is None or not si.on_wait:
                continue
            if any(w.ant_name and w.ant_name.startswith("DMASW") for w in si.on_wait):
                si.on_wait = [
                    w
                    for w in si.on_wait
                    if not (w.ant_name and w.ant_name.startswith("DMASW"))
                ]

    # Tile materializes the trigger's scale deps as standalone Pool event-sem
    # waits that serialize before the trigger's decode; fold them into the
    # trigger instruction so the decode overlaps the waiting (identical
    # ordering semantics, just fewer serialized sequencer steps)
    for block in nc.m.functions[0].blocks:
        insts = list(block.instructions)
        for idx, ins in enumerate(insts):
            if type(ins).__name__ != "InstTriggerDma" or ins.sync_info is None:
                continue
            j = idx - 1
            while j >= 0:
                p = insts[j]
                psi = p.sync_info
                if (
                    type(p).__name__ == "InstEventSemaphore"
                    and p.engine == mybir.EngineType.Pool
                    and psi is not None
                    and not psi.on_update
                    and psi.on_wait
                ):
                    ins.sync_info.on_wait = list(psi.on_wait) + list(
                        ins.sync_info.on_wait
                    )
                    psi.on_wait = []
                    j -= 1
                else:
                    break

    nc.compile()
    _CACHE["nc"] = nc
    return nc


def kernel(pixel_coords, attn_weights, in_frame_mask, log_sigma, **kwargs):
    pixel_coords = np.asarray(pixel_coords, dtype=np.float32)
    attn_weights = np.asarray(attn_weights, dtype=np.float32)
    mask_f = np.asarray(in_frame_mask).astype(np.float32)
    ls = float(np.asarray(log_sigma, dtype=np.float32))

    nc = _build()
    from concourse.bass_utils import run_bass_kernel_spmd

    in_maps = []
    for i in range(NCORES):
        sl = slice(i * BPC, (i + 1) * BPC)
        pc = pixel_coords[sl]  # (BPC, N, 2)
        aw = attn_weights[sl]  # (BPC, N)
        mf = mask_f[sl]
        pkt = np.zeros((N, 12), dtype=np.float32)
        pkt[:, 0] = pc[0, :, 0]
        pkt[:, 1] = pc[1, :, 0]
        pkt[:, 2] = pc[0, :, 1]
        pkt[:, 3] = pc[1, :, 1]
        pkt[:, 4] = aw[0]
        pkt[:, 5] = aw[1]
        pkt[:, 6] = mf[0]
        pkt[:, 7] = mf[1]
        pkt[:, 8] = ls
        in_maps.append({"pk": pkt})
    res = run_bass_kernel_spmd(nc, in_maps, core_ids=list(range(NCORES)))
    return np.concatenate(
        [r["out"].reshape(BPC, HP, W)[:, :H, :] for r in res.results], axis=0
    )

